# revision 12
# baseline (speedup 1.0000x reference)
"""Sparse ConvTranspose3d (gather + GEMM + scatter-add) on 8 TRN2 NeuronCores.

Sharding: active voxels (N dim) sorted spatially, split across 8 cores by the
output-row range their contributions land in; each core GEMMs its point shard
against all 27 kernel offsets and scatter-adds rows into its own (halo-padded)
output slab via the Ant dma_scatter_add instruction; host sums halo overlaps.

Key structure exploited: a point's three dz-offsets within one (dx,dy) family
always land on exactly consecutive output rows (their hash keys are adjacent
integers, all occupied), so the 27 per-point scatter tokens collapse into 9
three-row tokens (elem_size=192, elem_step=64) — a 3x cut in the Q7
descriptor-generation work that bounds this kernel.

Token spans within one instruction must not overlap (concurrent DMA RMW adds
would race). Same-family overlap happens exactly for input-voxel pairs at
z-distance 1 (rank delta exactly 2 in every family); the later point of each
such pair is extracted into a small "B" stream scattered by fixed 24576-row
windows after the main stream. All other concurrency hazards are removed by
round-robining families over 4 colored output slabs (separate DRAM tensors,
4 SWDGE queues): different colors never share a tensor, same-color
instructions are serialized by the framework's range-based dependency
tracking.

Bias is folded into the GEMM via 27 extra contraction rows (one-hot per-offset
"first contribution of this output row" masks); empty output rows get bias on
the host merge (placement of an input vector, like the halo merge itself).
"""
import numpy as np
import ml_dtypes

import concourse.bass as bass
import concourse.bacc as bacc
import concourse.tile as tile
import concourse.mybir as mybir
from concourse.bass_utils import run_bass_kernel_spmd

N_CORES = 8
KV = 27
NF = 9                           # (dx,dy) families, 3 dz rows each
FEL = 192                        # token payload: 3 rows x 64 = 192 f32
CIN = 64
COUT = 64
N_OUT = 1620000
SLAB = N_OUT // N_CORES          # 202500
MARGIN = 8192                    # halo rows on each side of a core's slab
SC_PTS = 896                     # points per scatter instruction (7 chunks)
CPS = SC_PTS // 128              # chunks per superchunk
IDXW = SC_PTS // 16
KAUG = CIN + KV                  # 91 contraction rows (feats + firstmask)
WCOLS = KV * COUT                # 1728
NCOLOR = 4                       # colored output slabs / SWDGE queues
BWIN = 24576                     # fixed window stride for the B stream
BSLOT = 128                      # point slots per B chunk
PHYS_ROWS = 2 * MARGIN + SLAB    # physical slab rows incl. halo
NW = -(-(PHYS_ROWS + 4) // BWIN)  # 9 fixed B windows
WORK_ROWS = PHYS_ROWS + 32776    # window slice + 3-row token slack
NBCOL = NF * NW * BSLOT          # B-stream ft columns

_prog_cache = {}


def _build_program(NSC, bases):
    """Build the SPMD Bass program (same for all cores)."""
    NPTS = NSC * SC_PTS
    nc = bacc.Bacc("TRN2", target_bir_lowering=False, debug=False,
                   enable_asserts=False, num_devices=N_CORES,
                   dynamic_dma_scratch_size=65536, num_swdge_queues=NCOLOR)
    ft = nc.dram_tensor("ft", [KAUG, NPTS], mybir.dt.bfloat16, kind="ExternalInput")
    wt = nc.dram_tensor("wt", [KAUG, WCOLS], mybir.dt.bfloat16, kind="ExternalInput")
    idx = nc.dram_tensor("idx", [NSC, 128, NF * IDXW], mybir.dt.int16,
                         kind="ExternalInput")
    ftb = nc.dram_tensor("ftb", [KAUG, NBCOL], mybir.dt.bfloat16,
                         kind="ExternalInput")
    idxb = nc.dram_tensor("idxb", [128, NF * NW * (BSLOT // 16)], mybir.dt.int16,
                          kind="ExternalInput")
    works = [nc.dram_tensor(f"work{c}", [WORK_ROWS, COUT], mybir.dt.float32,
                            kind="ExternalOutput") for c in range(NCOLOR)]

    def win_ap(col, base):
        return bass.AP(works[col][:].tensor, int(base) * COUT,
                       [[COUT, 32768], [1, FEL]])

    with tile.TileContext(nc) as tc:
        with (
            tc.tile_pool(name="const", bufs=1) as cpool,
            tc.tile_pool(name="cbuf", bufs=2) as cbpool,
            tc.tile_pool(name="ipool", bufs=3) as ipool,
            tc.tile_pool(name="bbuf", bufs=3) as bbpool,
        ):
            ft_t = cpool.tile([KAUG, NPTS], mybir.dt.bfloat16)
            wt_t = cpool.tile([KAUG, WCOLS], mybir.dt.bfloat16)
            ftb_t = cpool.tile([KAUG, NBCOL], mybir.dt.bfloat16)
            idxb_t = cpool.tile([128, NF * NW * (BSLOT // 16)], mybir.dt.int16)
            nc.sync.dma_start(out=ft_t[:], in_=ft[:])
            nc.sync.dma_start(out=wt_t[:], in_=wt[:])
            nc.sync.dma_start(out=ftb_t[:], in_=ftb[:])
            nc.sync.dma_start(out=idxb_t[:], in_=idxb[:])

            ppool_cm = tc.tile_pool(name="psum", bufs=2, space="PSUM")
            ppool = ppool_cm.__enter__()
            for sc in range(NSC):
                c_t = cbpool.tile([128, NF, CPS, FEL], mybir.dt.float32)
                i_t = ipool.tile([128, NF * IDXW], mybir.dt.int16)
                nc.sync.dma_start(out=i_t[:], in_=idx[sc])
                for ci in range(CPS):
                    ch = sc * CPS + ci
                    ps = ppool.tile([128, WCOLS], mybir.dt.float32, space="PSUM")
                    for mm in range(4):
                        n0 = mm * 512
                        n1 = min(n0 + 512, WCOLS)
                        nc.tensor.matmul(
                            out=ps[:, n0:n1],
                            lhsT=ft_t[:, ch * 128:(ch + 1) * 128],
                            rhs=wt_t[:, n0:n1],
                            start=True, stop=True)
                    if ci % 2 == 0:
                        nc.vector.tensor_copy(
                            out=c_t[:, :, ci, :],
                            in_=ps[:].rearrange("p (f e) -> p f e", e=FEL))
                    else:
                        nc.scalar.copy(
                            out=c_t[:, :, ci, :],
                            in_=ps[:].rearrange("p (f e) -> p f e", e=FEL))
                for f in range(NF):
                    nc.gpsimd.dma_scatter_add(
                        win_ap(f % NCOLOR, bases[sc * NF + f]),
                        c_t[:, f],
                        i_t[:, f * IDXW:(f + 1) * IDXW],
                        SC_PTS, SC_PTS, FEL, elem_step=COUT,
                        queue_num=f % NCOLOR)

            ppool_cm.__exit__(None, None, None)
            # B stream: z-conflict points, fixed 24576-row windows
            ppoolb_cm = tc.tile_pool(name="psumb", bufs=2, space="PSUM")
            ppoolb = ppoolb_cm.__enter__()
            for f in range(NF):
                for w in range(NW):
                    cwi = f * NW + w
                    psb = ppoolb.tile([128, FEL], mybir.dt.float32, space="PSUM")
                    nc.tensor.matmul(
                        out=psb[:],
                        lhsT=ftb_t[:, cwi * BSLOT:(cwi + 1) * BSLOT],
                        rhs=wt_t[:, f * FEL:(f + 1) * FEL],
                        start=True, stop=True)
                    cb = bbpool.tile([128, 1, FEL], mybir.dt.float32)
                    if (f + w) % 2 == 0:
                        nc.vector.tensor_copy(out=cb[:, 0, :], in_=psb[:])
                    else:
                        nc.scalar.copy(out=cb[:, 0, :], in_=psb[:])
                    nc.gpsimd.dma_scatter_add(
                        win_ap(f % NCOLOR, w * BWIN),
                        cb[:],
                        idxb_t[:, cwi * (BSLOT // 16):(cwi + 1) * (BSLOT // 16)],
                        BSLOT, BSLOT, FEL, elem_step=COUT,
                        queue_num=f % NCOLOR)
            ppoolb_cm.__exit__(None, None, None)
    nc.compile()
    return nc


def _wrap16(vals, cap):
    """int16 idx layout: token i at [i%16, i//16], replicated to 128 partitions."""
    a = np.zeros(cap, np.int16)
    a[:len(vals)] = vals
    blk = a.reshape(cap // 16, 16).T            # [16, cap/16]
    return np.tile(blk, (8, 1))                 # [128, cap/16]


def kernel(feats, weight, bias, out_index, n_out):
    feats = np.asarray(feats, np.float32)
    weight = np.asarray(weight, np.float32)
    bias = np.asarray(bias, np.float32)
    oi = np.asarray(out_index, np.int32)

    # ---- sort points spatially; merge duplicate-coordinate points ----
    order = np.argsort(oi[0], kind="stable")
    b0 = oi[0][order]
    dup = np.zeros(len(order), bool)
    dup[1:] = b0[1:] == b0[:-1]
    heads = np.where(~dup, np.arange(len(order)), 0)
    np.maximum.accumulate(heads, out=heads)
    f_s = feats[order].copy()
    if dup.any():
        np.add.at(f_s, heads[dup], f_s[np.flatnonzero(dup)])
    keep = ~dup
    f_s = f_s[keep]
    oi_s = oi[:, order[keep]]                   # [27, M] sorted, deduped
    M = oi_s.shape[1]

    # dz rows must be exactly consecutive within each family
    for f in range(NF):
        assert np.all(oi_s[3 * f + 1] == oi_s[3 * f] + 1)
        assert np.all(oi_s[3 * f + 2] == oi_s[3 * f] + 2)

    # ---- first-contribution mask (bias exactly once per non-empty row) ----
    flat = oi_s.reshape(-1)
    uniq, first = np.unique(flat, return_index=True)
    fm = np.zeros(KV * M, np.float32)
    fm[first] = 1.0
    fm = fm.reshape(KV, M)
    occupied = np.zeros(n_out, bool)
    occupied[uniq] = True
    empties = np.flatnonzero(~occupied)

    aug = np.concatenate([f_s.T, fm], 0).astype(ml_dtypes.bfloat16)  # [91, M]

    # ---- assign points to cores by the slab their center-offset row hits ----
    core_of = np.minimum(oi_s[KV // 2] // SLAB, N_CORES - 1)

    # ---- per-core: split off B points (z-neighbor conflicts), chunk ----
    mains, bs, phys_m = [], [], []
    for c in range(N_CORES):
        p = np.flatnonzero(core_of == c)
        r0 = oi_s[0, p]                         # family-0 dz0 rows, sorted
        bmask = np.zeros(len(p), bool)
        last_end = -10
        for i, r in enumerate(r0):
            if r < last_end:
                bmask[i] = True                 # token overlaps a kept token
            else:
                last_end = r + 3
        mains.append(p[~bmask])
        bs.append(p[bmask])
        phys_m.append(oi_s[:, p[~bmask]] - c * SLAB + MARGIN)
    counts = np.array([len(m) for m in mains])
    NSC = int(np.ceil(counts.max() / SC_PTS))
    NPTS = NSC * SC_PTS

    # per-(sc,family) window bases: min over cores of the run's rows
    bases = np.zeros(NSC * NF, np.int64)
    for sc in range(NSC):
        lo, hi = sc * SC_PTS, (sc + 1) * SC_PTS
        for f in range(NF):
            mn, mx = WORK_ROWS, 0
            for c in range(N_CORES):
                seg = phys_m[c][3 * f, lo:min(hi, counts[c])]
                if len(seg):
                    mn = min(mn, seg.min())
                    mx = max(mx, seg.max())
            if mx == 0 and mn == WORK_ROWS:
                mn, mx = 0, 0
            assert mx - mn <= 32760, f"window span {mx-mn} at sc={sc} f={f}"
            bases[sc * NF + f] = mn

    key = (NSC, tuple(bases))
    if key not in _prog_cache:
        _prog_cache[key] = _build_program(NSC, bases)
    nc = _prog_cache[key]

    # ---- per-core input arrays ----
    wt_aug = np.zeros((KAUG, WCOLS), np.float32)
    for k in range(KV):
        wt_aug[:CIN, k * COUT:(k + 1) * COUT] = weight[k].T
        wt_aug[CIN + k, k * COUT:(k + 1) * COUT] = bias
    wt_aug = wt_aug.astype(ml_dtypes.bfloat16)

    in_maps = []
    for c in range(N_CORES):
        m = mains[c]
        cnt = len(m)
        ft_np = np.zeros((KAUG, NPTS), ml_dtypes.bfloat16)
        ft_np[:, :cnt] = aug[:, m]
        idx_np = np.zeros((NSC, 128, NF * IDXW), np.int16)
        for sc in range(NSC):
            lo = sc * SC_PTS
            hi = max(lo, min(lo + SC_PTS, cnt))
            for f in range(NF):
                base = bases[sc * NF + f]
                if hi > lo:
                    offs = phys_m[c][3 * f, lo:hi] - base
                    pad = offs.max() + 3
                    if pad > 32765:
                        pad = offs.min() - 3
                    assert 0 <= pad <= 32765
                else:
                    offs = np.zeros(0, np.int64)
                    pad = 0
                full = np.full(SC_PTS, pad, np.int64)
                full[:hi - lo] = offs
                idx_np[sc, :, f * IDXW:(f + 1) * IDXW] = _wrap16(
                    full.astype(np.int16), SC_PTS)
        # B stream: fixed windows per family
        ftb_np = np.zeros((KAUG, NBCOL), ml_dtypes.bfloat16)
        idxb_np = np.zeros((128, NF * NW * (BSLOT // 16)), np.int16)
        bp = bs[c]
        if len(bp):
            phb = oi_s[:, bp] - c * SLAB + MARGIN
        for f in range(NF):
            if not len(bp):
                continue
            rows = phb[3 * f]
            ws = rows // BWIN
            for w in range(NW):
                sel = np.flatnonzero(ws == w)
                assert len(sel) <= BSLOT, f"B overflow {len(sel)} at f={f} w={w}"
                if not len(sel):
                    continue
                cwi = f * NW + w
                ftb_np[:, cwi * BSLOT:cwi * BSLOT + len(sel)] = aug[:, bp[sel]]
                offs = rows[sel] - w * BWIN
                pad = offs.max() + 3
                full = np.full(BSLOT, pad, np.int64)
                full[:len(sel)] = offs
                idxb_np[:, cwi * (BSLOT // 16):(cwi + 1) * (BSLOT // 16)] = \
                    _wrap16(full.astype(np.int16), BSLOT)
        in_maps.append({"ft": ft_np, "wt": wt_aug, "idx": idx_np,
                        "ftb": ftb_np, "idxb": idxb_np})

    res = run_bass_kernel_spmd(nc, in_maps, list(range(N_CORES)))

    # ---- merge halo-overlapped, color-split slabs ----
    out = np.zeros((n_out, COUT), np.float32)
    for c in range(N_CORES):
        lo = c * SLAB - MARGIN
        g0, g1 = max(0, lo), min(int(n_out), (c + 1) * SLAB + MARGIN)
        for col in range(NCOLOR):
            sl = res.results[c][f"work{col}"]
            out[g0:g1] += sl[g0 - lo:g1 - lo]
    out[empties] = bias            # bias-only rows: place the input vector
    return out


# revision 15
# speedup vs baseline: 1.5641x; 1.5641x over previous
"""Sparse ConvTranspose3d (gather + GEMM + scatter-add) on 8 TRN2 NeuronCores.

Sharding: active voxels (N dim) sorted spatially, split across 8 cores by the
output-row range their contributions land in; each core GEMMs its point shard
against all 27 kernel offsets and scatter-adds rows into its own (halo-padded)
output slab via the Ant dma_scatter_add instruction; host sums halo overlaps.

Key structure exploited: a point's three dz-offsets within one (dx,dy) family
always land on exactly consecutive output rows (their hash keys are adjacent
integers, all occupied), so the 27 per-point scatter tokens collapse into 9
three-row tokens (elem_size=192, elem_step=64) — a 3x cut in the Q7
descriptor-generation work that bounds this kernel.

Token spans within one instruction must not overlap (concurrent DMA RMW adds
would race). Same-family overlap happens exactly for input-voxel pairs at
z-distance 1 (rank delta exactly 2 in every family); the later point of each
such pair is extracted into a small "B" stream scattered by fixed 24576-row
windows after the main stream. All other concurrency hazards are removed by
round-robining families over 4 colored output slabs (separate DRAM tensors,
4 SWDGE queues): different colors never share a tensor, same-color
instructions are serialized by the framework's range-based dependency
tracking.

Bias is folded into the GEMM via 27 extra contraction rows (one-hot per-offset
"first contribution of this output row" masks); empty output rows get bias on
the host merge (placement of an input vector, like the halo merge itself).
"""
import numpy as np
import ml_dtypes

import concourse.bass as bass
import concourse.bacc as bacc
import concourse.tile as tile
import concourse.mybir as mybir
from concourse.bass_utils import run_bass_kernel_spmd

N_CORES = 8
KV = 27
NF = 9                           # (dx,dy) families, 3 dz rows each
FEL = 192                        # token payload: 3 rows x 64 = 192 f32
CIN = 64
COUT = 64
N_OUT = 1620000
SLAB = N_OUT // N_CORES          # 202500
MARGIN = 8192                    # halo rows on each side of a core's slab
SC_PTS = 896                     # points per scatter instruction (7 chunks)
CPS = SC_PTS // 128              # chunks per superchunk
IDXW = SC_PTS // 16
KAUG = CIN + KV                  # 91 contraction rows (feats + firstmask)
WCOLS = KV * COUT                # 1728
NCOLOR = 4                       # colored output slabs / SWDGE queues
BWIN = 31744                     # fixed window stride for the B stream
BSLOT = 128                      # point slots per B chunk
PHYS_ROWS = 2 * MARGIN + SLAB    # physical slab rows incl. halo
NW = -(-(PHYS_ROWS + 4) // BWIN)  # 9 fixed B windows
WORK_ROWS = PHYS_ROWS + 32776    # window slice + 3-row token slack
NBCOL = NF * NW * BSLOT          # B-stream ft columns

_prog_cache = {}


def _build_program(NSC, bases, bchunks):
    """Build the SPMD Bass program (same for all cores)."""
    NPTS = NSC * SC_PTS
    nc = bacc.Bacc("TRN2", target_bir_lowering=False, debug=False,
                   enable_asserts=False, num_devices=N_CORES,
                   dynamic_dma_scratch_size=65536, num_swdge_queues=NCOLOR)
    ft = nc.dram_tensor("ft", [KAUG, NPTS], mybir.dt.bfloat16, kind="ExternalInput")
    wt = nc.dram_tensor("wt", [KAUG, WCOLS], mybir.dt.bfloat16, kind="ExternalInput")
    idx = nc.dram_tensor("idx", [NSC, 128, NF * IDXW], mybir.dt.int16,
                         kind="ExternalInput")
    ftb = nc.dram_tensor("ftb", [KAUG, NBCOL], mybir.dt.bfloat16,
                         kind="ExternalInput")
    idxb = nc.dram_tensor("idxb", [128, NF * NW * (BSLOT // 16)], mybir.dt.int16,
                          kind="ExternalInput")
    works = [nc.dram_tensor(f"work{c}", [WORK_ROWS, COUT], mybir.dt.float32,
                            kind="ExternalOutput") for c in range(NCOLOR)]

    def win_ap(col, base):
        return bass.AP(works[col][:].tensor, int(base) * COUT,
                       [[COUT, 32768], [1, FEL]])

    with tile.TileContext(nc) as tc:
        with (
            tc.tile_pool(name="const", bufs=1) as cpool,
            tc.tile_pool(name="cbuf", bufs=2) as cbpool,
            tc.tile_pool(name="ipool", bufs=3) as ipool,
            tc.tile_pool(name="bbuf", bufs=3) as bbpool,
        ):
            ft_t = cpool.tile([KAUG, NPTS], mybir.dt.bfloat16)
            wt_t = cpool.tile([KAUG, WCOLS], mybir.dt.bfloat16)
            ftb_t = cpool.tile([KAUG, NBCOL], mybir.dt.bfloat16)
            idxb_t = cpool.tile([128, NF * NW * (BSLOT // 16)], mybir.dt.int16)
            nc.sync.dma_start(out=ft_t[:], in_=ft[:])
            nc.sync.dma_start(out=wt_t[:], in_=wt[:])
            nc.sync.dma_start(out=ftb_t[:], in_=ftb[:])
            nc.sync.dma_start(out=idxb_t[:], in_=idxb[:])

            ppool_cm = tc.tile_pool(name="psum", bufs=2, space="PSUM")
            ppool = ppool_cm.__enter__()

            def get_ps():
                return ppool.tile([128, WCOLS], mybir.dt.float32, space="PSUM",
                                  name="ps")

            for sc in range(NSC):
                c_t = cbpool.tile([128, NF, CPS, FEL], mybir.dt.float32)
                i_t = ipool.tile([128, NF * IDXW], mybir.dt.int16)
                nc.sync.dma_start(out=i_t[:], in_=idx[sc])
                for ci in range(CPS):
                    ch = sc * CPS + ci
                    ps = get_ps()
                    for mm in range(4):
                        n0 = mm * 512
                        n1 = min(n0 + 512, WCOLS)
                        nc.tensor.matmul(
                            out=ps[:, n0:n1],
                            lhsT=ft_t[:, ch * 128:(ch + 1) * 128],
                            rhs=wt_t[:, n0:n1],
                            start=True, stop=True)
                    if ci % 2 == 0:
                        nc.vector.tensor_copy(
                            out=c_t[:, :, ci, :],
                            in_=ps[:].rearrange("p (f e) -> p f e", e=FEL))
                    else:
                        nc.scalar.copy(
                            out=c_t[:, :, ci, :],
                            in_=ps[:].rearrange("p (f e) -> p f e", e=FEL))
                for f in range(NF):
                    nc.gpsimd.dma_scatter_add(
                        win_ap(f % NCOLOR, bases[sc * NF + f]),
                        c_t[:, f],
                        i_t[:, f * IDXW:(f + 1) * IDXW],
                        SC_PTS, SC_PTS, FEL, elem_step=COUT,
                        queue_num=f % NCOLOR)

            # B stream: z-conflict points, fixed windows, window-major order
            for w in range(NW):
                for f in range(NF):
                    if (f, w) not in bchunks:
                        continue
                    cwi = f * NW + w
                    psb = get_ps()
                    nc.tensor.matmul(
                        out=psb[:, 0:FEL],
                        lhsT=ftb_t[:, cwi * BSLOT:(cwi + 1) * BSLOT],
                        rhs=wt_t[:, f * FEL:(f + 1) * FEL],
                        start=True, stop=True)
                    cb = bbpool.tile([128, 1, FEL], mybir.dt.float32)
                    if (f + w) % 2 == 0:
                        nc.vector.tensor_copy(out=cb[:, 0, :], in_=psb[:, 0:FEL])
                    else:
                        nc.scalar.copy(out=cb[:, 0, :], in_=psb[:, 0:FEL])
                    nc.gpsimd.dma_scatter_add(
                        win_ap(f % NCOLOR, w * BWIN),
                        cb[:],
                        idxb_t[:, cwi * (BSLOT // 16):(cwi + 1) * (BSLOT // 16)],
                        BSLOT, BSLOT, FEL, elem_step=COUT,
                        queue_num=f % NCOLOR)
            ppool_cm.__exit__(None, None, None)
    nc.compile()
    return nc


def _wrap16(vals, cap):
    """int16 idx layout: token i at [i%16, i//16], replicated to 128 partitions."""
    a = np.zeros(cap, np.int16)
    a[:len(vals)] = vals
    blk = a.reshape(cap // 16, 16).T            # [16, cap/16]
    return np.tile(blk, (8, 1))                 # [128, cap/16]


def kernel(feats, weight, bias, out_index, n_out):
    feats = np.asarray(feats, np.float32)
    weight = np.asarray(weight, np.float32)
    bias = np.asarray(bias, np.float32)
    oi = np.asarray(out_index, np.int32)

    # ---- sort points spatially; merge duplicate-coordinate points ----
    order = np.argsort(oi[0], kind="stable")
    b0 = oi[0][order]
    dup = np.zeros(len(order), bool)
    dup[1:] = b0[1:] == b0[:-1]
    heads = np.where(~dup, np.arange(len(order)), 0)
    np.maximum.accumulate(heads, out=heads)
    f_s = feats[order].copy()
    if dup.any():
        np.add.at(f_s, heads[dup], f_s[np.flatnonzero(dup)])
    keep = ~dup
    f_s = f_s[keep]
    oi_s = oi[:, order[keep]]                   # [27, M] sorted, deduped
    M = oi_s.shape[1]

    # dz rows must be exactly consecutive within each family
    for f in range(NF):
        assert np.all(oi_s[3 * f + 1] == oi_s[3 * f] + 1)
        assert np.all(oi_s[3 * f + 2] == oi_s[3 * f] + 2)

    # ---- first-contribution mask (bias exactly once per non-empty row) ----
    flat = oi_s.reshape(-1)
    uniq, first = np.unique(flat, return_index=True)
    fm = np.zeros(KV * M, np.float32)
    fm[first] = 1.0
    fm = fm.reshape(KV, M)
    occupied = np.zeros(n_out, bool)
    occupied[uniq] = True
    empties = np.flatnonzero(~occupied)

    aug = np.concatenate([f_s.T, fm], 0).astype(ml_dtypes.bfloat16)  # [91, M]

    # ---- assign points to cores by the slab their center-offset row hits ----
    core_of = np.minimum(oi_s[KV // 2] // SLAB, N_CORES - 1)

    # ---- per-core: split off B points (z-neighbor conflicts), chunk ----
    mains, bs, phys_m = [], [], []
    for c in range(N_CORES):
        p = np.flatnonzero(core_of == c)
        r0 = oi_s[0, p]                         # family-0 dz0 rows, sorted
        bmask = np.zeros(len(p), bool)
        last_end = -10
        for i, r in enumerate(r0):
            if r < last_end:
                bmask[i] = True                 # token overlaps a kept token
            else:
                last_end = r + 3
        mains.append(p[~bmask])
        bs.append(p[bmask])
        phys_m.append(oi_s[:, p[~bmask]] - c * SLAB + MARGIN)
    counts = np.array([len(m) for m in mains])
    NSC = int(np.ceil(counts.max() / SC_PTS))
    NPTS = NSC * SC_PTS

    # per-(sc,family) window bases: min over cores of the run's rows
    bases = np.zeros(NSC * NF, np.int64)
    for sc in range(NSC):
        lo, hi = sc * SC_PTS, (sc + 1) * SC_PTS
        for f in range(NF):
            mn, mx = WORK_ROWS, 0
            for c in range(N_CORES):
                seg = phys_m[c][3 * f, lo:min(hi, counts[c])]
                if len(seg):
                    mn = min(mn, seg.min())
                    mx = max(mx, seg.max())
            if mx == 0 and mn == WORK_ROWS:
                mn, mx = 0, 0
            assert mx - mn <= 32760, f"window span {mx-mn} at sc={sc} f={f}"
            bases[sc * NF + f] = mn

    bchunks = set()
    for c in range(N_CORES):
        bp = bs[c]
        if not len(bp):
            continue
        phb = oi_s[:, bp] - c * SLAB + MARGIN
        for f in range(NF):
            for w in np.unique(phb[3 * f] // BWIN):
                bchunks.add((f, int(w)))
    bchunks = frozenset(bchunks)
    key = (NSC, tuple(bases), bchunks)
    if key not in _prog_cache:
        _prog_cache[key] = _build_program(NSC, bases, bchunks)
    nc = _prog_cache[key]

    # ---- per-core input arrays ----
    wt_aug = np.zeros((KAUG, WCOLS), np.float32)
    for k in range(KV):
        wt_aug[:CIN, k * COUT:(k + 1) * COUT] = weight[k].T
        wt_aug[CIN + k, k * COUT:(k + 1) * COUT] = bias
    wt_aug = wt_aug.astype(ml_dtypes.bfloat16)

    in_maps = []
    for c in range(N_CORES):
        m = mains[c]
        cnt = len(m)
        ft_np = np.zeros((KAUG, NPTS), ml_dtypes.bfloat16)
        ft_np[:, :cnt] = aug[:, m]
        idx_np = np.zeros((NSC, 128, NF * IDXW), np.int16)
        for sc in range(NSC):
            lo = sc * SC_PTS
            hi = max(lo, min(lo + SC_PTS, cnt))
            for f in range(NF):
                base = bases[sc * NF + f]
                if hi > lo:
                    offs = phys_m[c][3 * f, lo:hi] - base
                    pad = offs.max() + 3
                    if pad > 32765:
                        pad = offs.min() - 3
                    assert 0 <= pad <= 32765
                else:
                    offs = np.zeros(0, np.int64)
                    pad = 0
                full = np.full(SC_PTS, pad, np.int64)
                full[:hi - lo] = offs
                idx_np[sc, :, f * IDXW:(f + 1) * IDXW] = _wrap16(
                    full.astype(np.int16), SC_PTS)
        # B stream: fixed windows per family
        ftb_np = np.zeros((KAUG, NBCOL), ml_dtypes.bfloat16)
        idxb_np = np.zeros((128, NF * NW * (BSLOT // 16)), np.int16)
        bp = bs[c]
        if len(bp):
            phb = oi_s[:, bp] - c * SLAB + MARGIN
        for f in range(NF):
            if not len(bp):
                continue
            rows = phb[3 * f]
            ws = rows // BWIN
            for w in range(NW):
                sel = np.flatnonzero(ws == w)
                assert len(sel) <= BSLOT, f"B overflow {len(sel)} at f={f} w={w}"
                if not len(sel):
                    continue
                cwi = f * NW + w
                ftb_np[:, cwi * BSLOT:cwi * BSLOT + len(sel)] = aug[:, bp[sel]]
                offs = rows[sel] - w * BWIN
                pad = offs.max() + 3
                full = np.full(BSLOT, pad, np.int64)
                full[:len(sel)] = offs
                idxb_np[:, cwi * (BSLOT // 16):(cwi + 1) * (BSLOT // 16)] = \
                    _wrap16(full.astype(np.int16), BSLOT)
        in_maps.append({"ft": ft_np, "wt": wt_aug, "idx": idx_np,
                        "ftb": ftb_np, "idxb": idxb_np})

    res = run_bass_kernel_spmd(nc, in_maps, list(range(N_CORES)))

    # ---- merge halo-overlapped, color-split slabs ----
    out = np.zeros((n_out, COUT), np.float32)
    for c in range(N_CORES):
        lo = c * SLAB - MARGIN
        g0, g1 = max(0, lo), min(int(n_out), (c + 1) * SLAB + MARGIN)
        for col in range(NCOLOR):
            sl = res.results[c][f"work{col}"]
            out[g0:g1] += sl[g0 - lo:g1 - lo]
    out[empties] = bias            # bias-only rows: place the input vector
    return out
